# revision 9
# baseline (speedup 1.0000x reference)
"""Cutout kernel for Trainium2 (Bass/Tile), SPMD over 8 NeuronCores.

Problem: x [256,3,224,224] f32; cy, cx [1,256] i32 hole centers. Zero a
16x16 box (clipped to the image) centered at (cy, cx) in every channel of
each sample: out = x * (1 - in_y[h] * in_x[w]).

Strategy (pure data parallel, 32 samples/core):
  - Host precomputes per-sample 0/1 row/col indicator vectors iy [B,H],
    ix [B,W] from cy/cx (tiny integer work; the 147 MB tensor op runs on
    device).
  - Device, per half-image (rows g*112..g*112+111 of all 3 channels): one
    294 KB DMA loads a [112, 3*224] SBUF tile (partition p = row g*112+p).
    The [112, 224] mask  m[p, w] = 1 - iy[g*112+p]*ix[w]  is produced by a
    single K=2 PE matmul (outer-product trick: [1;iy]^T @ [1;-ix]) into
    PSUM, so the TensorEngine (otherwise idle) builds the mask. One DVE
    scalar_tensor_tensor computes out = m (broadcast over channels) * x,
    and one 294 KB DMA stores it.
  - Memory-bound roofline: 2 x 18.4 MB HBM traffic per core.
"""

import numpy as np

import concourse.bass as bass
import concourse.mybir as mybir
import concourse.tile as tile
from concourse.bass_utils import run_bass_kernel_spmd

N_CORES = 8
B, C, H, W = 256, 3, 224, 224
BPC = B // N_CORES  # samples per core
HALF = 8  # LENGTH // 2
G = 2  # row-halves per image
P = H // G  # 112 partitions
F32 = mybir.dt.float32


def legalize_waits(nc: bass.Bass, max_waits: int = 1) -> None:
    """This toolchain's walrus codegen rejects any instruction carrying more
    than one sync-wait command (including Tile's own kernel-tail Drain).
    Engine queues execute in order, so hoist extra waits onto standalone
    NoOps emitted just before the instruction on the same engine queue."""
    for f in nc.m.functions:
        for blk in f.blocks:
            out = []
            changed = False
            for ins in blk.instructions:
                si = ins.sync_info
                waits = list(si.on_wait) if si is not None and si.on_wait else []
                if len(waits) > max_waits:
                    changed = True
                    for k, w in enumerate(waits[:-max_waits]):
                        nop = mybir.InstNoOp(
                            name=f"{ins.name}-wsplit{k}", engine=ins.engine
                        )
                        nop.sync_info = mybir.SyncInfo(on_wait=[w], on_update=[])
                        out.append(nop)
                    ins.sync_info = mybir.SyncInfo(
                        on_wait=waits[-max_waits:], on_update=list(si.on_update or [])
                    )
                out.append(ins)
            if changed:
                blk.instructions = out


def build_nc(bpc: int = BPC, repeat: int = 1, legalize: bool = True) -> bass.Bass:
    """Build the SPMD per-core Bass program (identical on all cores)."""
    nc = bass.Bass()
    x_d = nc.declare_dram_parameter("x", [bpc, C, H, W], F32, isOutput=False)
    # aux = [lhs | rhs] concatenated on the free dim: one DMA -> one sync
    # wait on the PE LoadWeights (HW limit: it can't carry several).
    naux = bpc * G * P + bpc * W
    a_d = nc.declare_dram_parameter("aux", [2, naux], F32, isOutput=False)
    o_d = nc.declare_dram_parameter("out", [bpc, C, H, W], F32, isOutput=True)
    r_off = bpc * G * P

    with tile.TileContext(nc) as tc:
        with (
            tc.tile_pool(name="aux", bufs=1) as aux,
            tc.tile_pool(name="xin", bufs=4) as xin,
            tc.tile_pool(name="xout", bufs=4) as xout,
            tc.tile_pool(name="mpsum", bufs=4, space=bass.MemorySpace.PSUM) as mpsum,
        ):
            a_t = aux.tile([2, naux], F32)
            nc.sync.dma_start(out=a_t[:], in_=a_d[:])
            l_t = a_t[:, :r_off]
            r_t = a_t[:, r_off:]
            for _ in range(repeat):
                for s in range(bpc):
                    for g in range(G):
                        xt = xin.tile([P, C * W], F32, tag="xt")
                        nc.sync.dma_start(
                            out=xt[:].rearrange("p (c w) -> p c w", c=C),
                            in_=x_d[s][:, g * P : (g + 1) * P, :].rearrange(
                                "c p w -> p c w"
                            ),
                        )
                        m = mpsum.tile([P, W], F32, tag="m")
                        nc.tensor.matmul(
                            m[:],
                            l_t[:, (s * G + g) * P : (s * G + g + 1) * P],
                            r_t[:, s * W : (s + 1) * W],
                            start=True,
                            stop=True,
                        )
                        ot = xout.tile([P, C * W], F32, tag="ot")
                        nc.vector.scalar_tensor_tensor(
                            out=ot[:].rearrange("p (c w) -> p c w", c=C),
                            in0=m[:][:, None, :].broadcast_to([P, C, W]),
                            scalar=0.0,
                            in1=xt[:].rearrange("p (c w) -> p c w", c=C),
                            op0=mybir.AluOpType.bypass,
                            op1=mybir.AluOpType.mult,
                        )
                        nc.sync.dma_start(
                            out=o_d[s][:, g * P : (g + 1) * P, :].rearrange(
                                "c p w -> p c w"
                            ),
                            in_=ot[:].rearrange("p (c w) -> p c w", c=C),
                        )
    if legalize:
        legalize_waits(nc)
    return nc


def make_aux(cy: np.ndarray, cx: np.ndarray, n_cores: int = N_CORES):
    """Host-side: per-core lhs [2, bpc*G*P] / rhs [2, bpc*W] f32 arrays."""
    b = cy.shape[1]
    bpc = b // n_cores
    cy0 = cy[0].astype(np.int64)
    cx0 = cx[0].astype(np.int64)
    ys = np.arange(H, dtype=np.int64)
    xs = np.arange(W, dtype=np.int64)
    iy = (
        (ys[None, :] >= (cy0[:, None] - HALF)) & (ys[None, :] < (cy0[:, None] + HALF))
    ).astype(np.float32)  # [B, H]
    ixm = (
        (xs[None, :] >= (cx0[:, None] - HALF)) & (xs[None, :] < (cx0[:, None] + HALF))
    ).astype(np.float32)  # [B, W]

    lhs = np.ones((n_cores, 2, bpc * G * P), np.float32)
    lhs[:, 1] = iy.reshape(n_cores, bpc * G * P)

    rhs = np.ones((n_cores, 2, bpc * W), np.float32)
    rhs[:, 1] = -ixm.reshape(n_cores, bpc * W)
    return np.concatenate([lhs, rhs], axis=2)


_NC_CACHE: dict = {}


def kernel(x: np.ndarray, cy: np.ndarray, cx: np.ndarray) -> np.ndarray:
    assert x.shape == (B, C, H, W) and x.dtype == np.float32
    nc = _NC_CACHE.get("nc")
    if nc is None:
        nc = _NC_CACHE["nc"] = build_nc()
    aux = make_aux(cy, cx)
    xs = np.ascontiguousarray(x).reshape(N_CORES, BPC, C, H, W)
    in_maps = [{"x": xs[i], "aux": aux[i]} for i in range(N_CORES)]
    res = run_bass_kernel_spmd(nc, in_maps, list(range(N_CORES)))
    out = np.concatenate([res.results[i]["out"] for i in range(N_CORES)], axis=0)
    return out.reshape(B, C, H, W)


# revision 11
# speedup vs baseline: 1.3589x; 1.3589x over previous
"""Cutout kernel for Trainium2 (Bass/Tile), SPMD over 8 NeuronCores.

Problem: x [256,3,224,224] f32; cy, cx [1,256] i32 hole centers. Zero a
16x16 box (clipped to the image) per sample across all channels:
out = x * (1 - in_y[h] * in_x[w]).

Strategy (pure data parallel, 32 samples/core, no collectives):
  - Host precomputes per-sample 0/1 row/col indicator vectors iy [B,H],
    ix [B,W] from cy/cx (trivial integer work; the 147 MB tensor op runs
    on device).
  - Layout: flatten (c,h) -> 672 rows; partition p in [0,112) holds rows
    6p..6p+5, so every DMA moves 5376B-contiguous per-partition segments;
    4 samples are coalesced per ~2.3 MB DMA transfer.
  - Mask m[p, j*224+w] = 1 - iy[(6p+j)%224]*ix[w] is built on the
    (otherwise idle) TensorEngine: 6 tiny K=2 bf16 outer-product matmuls
    per sample ([1; iy_j]^T @ [1; -ix]) into PSUM (bank-aligned slices).
  - One DVE scalar_tensor_tensor per sample multiplies the [112,1344]
    image tile by the mask; DMA out.
  - Memory-bound: 2 x 18.4 MB HBM traffic per core (~roofline).

This toolchain's walrus codegen rejects instructions carrying >1 sync
wait, so legalize_waits() hoists extra waits onto same-engine NoOps
(engine queues are in-order, preserving semantics).
"""

import numpy as np
import ml_dtypes

import concourse.bass as bass
import concourse.mybir as mybir
import concourse.tile as tile
from concourse.bass_utils import run_bass_kernel_spmd

N_CORES = 8
B, C, H, W = 256, 3, 224, 224
BPC = B // N_CORES  # samples per core
HALF = 8  # LENGTH // 2
F32 = mybir.dt.float32
BF16 = mybir.dt.bfloat16
P = 112             # partitions
RPP = (C * H) // P  # rows per partition = 6
FS = RPP * W        # free elems per sample = 1344
SG = 4              # samples per DMA group
BUFS = 2


def legalize_waits(nc: bass.Bass, max_waits: int = 1) -> None:
    """Hoist extra sync waits onto standalone same-engine NoOps (this
    walrus build allows at most one sync-wait command per instruction)."""
    for f in nc.m.functions:
        for blk in f.blocks:
            out = []
            changed = False
            for ins in blk.instructions:
                si = ins.sync_info
                waits = list(si.on_wait) if si is not None and si.on_wait else []
                if len(waits) > max_waits:
                    changed = True
                    for k, w in enumerate(waits[:-max_waits]):
                        nop = mybir.InstNoOp(
                            name=f"{ins.name}-wsplit{k}", engine=ins.engine
                        )
                        nop.sync_info = mybir.SyncInfo(on_wait=[w], on_update=[])
                        out.append(nop)
                    ins.sync_info = mybir.SyncInfo(
                        on_wait=waits[-max_waits:], on_update=list(si.on_update or [])
                    )
                out.append(ins)
            if changed:
                blk.instructions = out


def build_nc(bpc: int = BPC, repeat: int = 1, legalize: bool = True,
             sg: int = SG, bufs: int = BUFS, dual_ring: bool = False) -> bass.Bass:
    """Build the SPMD per-core Bass program (identical on all cores)."""
    assert bpc % sg == 0
    nc = bass.Bass()
    x_d = nc.declare_dram_parameter("x", [bpc, C, H, W], F32, isOutput=False)
    l_d = nc.declare_dram_parameter("lhs", [2, bpc * RPP * P], BF16, isOutput=False)
    r_d = nc.declare_dram_parameter("rhs", [2, bpc * W], BF16, isOutput=False)
    o_d = nc.declare_dram_parameter("out", [bpc, C, H, W], F32, isOutput=True)

    with tile.TileContext(nc) as tc:
        with (
            tc.tile_pool(name="aux", bufs=1) as aux,
            tc.tile_pool(name="xin", bufs=bufs) as xin,
            tc.tile_pool(name="xout", bufs=bufs) as xout,
            tc.tile_pool(name="mpsum", bufs=2, space=bass.MemorySpace.PSUM) as mpsum,
        ):
            l_t = aux.tile([2, bpc * RPP * P], BF16)
            nc.sync.dma_start(out=l_t[:], in_=l_d[:])
            r_t = aux.tile([2, bpc * W], BF16)
            nc.sync.dma_start(out=r_t[:], in_=r_d[:])
            for _ in range(repeat):
                for s0 in range(0, bpc, sg):
                    xt = xin.tile([P, sg * FS], F32, tag="xt")
                    nc.sync.dma_start(
                        out=xt[:].rearrange("p (b q) -> p b q", b=sg),
                        in_=x_d[s0 : s0 + sg]
                        .rearrange("b c h w -> b (c h w)")
                        .rearrange("b (p q) -> p b q", p=P),
                    )
                    ot = xout.tile([P, sg * FS], F32, tag="ot")
                    BANK = 512  # f32 elems per PSUM bank
                    for i in range(sg):
                        s = s0 + i
                        # 2 j-slices (448 elems) per bank; matmul can't
                        # cross PSUM bank boundaries
                        m = mpsum.tile([P, (RPP // 2) * BANK], F32, tag="m")
                        for j in range(RPP):
                            off = (j // 2) * BANK + (j % 2) * W
                            nc.tensor.matmul(
                                m[:, off : off + W],
                                l_t[:, (s * RPP + j) * P : (s * RPP + j + 1) * P],
                                r_t[:, s * W : (s + 1) * W],
                                start=True,
                                stop=True,
                            )
                        nc.vector.scalar_tensor_tensor(
                            out=ot[:, i * FS : (i + 1) * FS].rearrange(
                                "p (k d) -> p k d", d=2 * W
                            ),
                            in0=m[:].rearrange("p (k d) -> p k d", d=BANK)[
                                :, :, : 2 * W
                            ],
                            scalar=0.0,
                            in1=xt[:, i * FS : (i + 1) * FS].rearrange(
                                "p (k d) -> p k d", d=2 * W
                            ),
                            op0=mybir.AluOpType.bypass,
                            op1=mybir.AluOpType.mult,
                        )
                    out_eng = nc.scalar if dual_ring else nc.sync
                    out_eng.dma_start(
                        out=o_d[s0 : s0 + sg]
                        .rearrange("b c h w -> b (c h w)")
                        .rearrange("b (p q) -> p b q", p=P),
                        in_=ot[:].rearrange("p (b q) -> p b q", b=sg),
                    )
    if legalize:
        legalize_waits(nc)
    return nc


def make_aux(cy: np.ndarray, cx: np.ndarray, n_cores: int = N_CORES):
    """Host-side: per-core bf16 stationary/moving mask factors.
    lhs [2, bpc*RPP*P]: row0 = 1, row1[(s*RPP+j)*P+p] = iy[b, (6p+j)%224].
    rhs [2, bpc*W]:     row0 = 1, row1[s*W+w] = -ix[b, w]."""
    b = cy.shape[1]
    bpc = b // n_cores
    cy0 = cy[0].astype(np.int64)
    cx0 = cx[0].astype(np.int64)
    ys = np.arange(H, dtype=np.int64)
    xs = np.arange(W, dtype=np.int64)
    iy = (
        (ys[None, :] >= (cy0[:, None] - HALF)) & (ys[None, :] < (cy0[:, None] + HALF))
    ).astype(np.float32)  # [B, H]
    ixm = (
        (xs[None, :] >= (cx0[:, None] - HALF)) & (xs[None, :] < (cx0[:, None] + HALF))
    ).astype(np.float32)  # [B, W]

    ps = np.arange(P)
    lhs = np.ones((n_cores, 2, bpc, RPP, P), np.float32)
    iyr = iy.reshape(n_cores, bpc, H)
    for j in range(RPP):
        hidx = (RPP * ps + j) % H  # [P]
        lhs[:, 1, :, j, :] = iyr[:, :, hidx]
    rhs = np.ones((n_cores, 2, bpc, W), np.float32)
    rhs[:, 1] = -ixm.reshape(n_cores, bpc, W)
    return (
        lhs.reshape(n_cores, 2, bpc * RPP * P).astype(ml_dtypes.bfloat16),
        rhs.reshape(n_cores, 2, bpc * W).astype(ml_dtypes.bfloat16),
    )


_NC_CACHE: dict = {}


def kernel(x: np.ndarray, cy: np.ndarray, cx: np.ndarray) -> np.ndarray:
    x = np.ascontiguousarray(np.asarray(x, dtype=np.float32))
    assert x.shape == (B, C, H, W)
    nc = _NC_CACHE.get("nc")
    if nc is None:
        nc = _NC_CACHE["nc"] = build_nc()
    lhs, rhs = make_aux(np.asarray(cy), np.asarray(cx))
    xs = x.reshape(N_CORES, BPC, C, H, W)
    in_maps = [
        {"x": xs[i], "lhs": lhs[i], "rhs": rhs[i]} for i in range(N_CORES)
    ]
    res = run_bass_kernel_spmd(nc, in_maps, list(range(N_CORES)))
    out = np.concatenate([res.results[i]["out"] for i in range(N_CORES)], axis=0)
    return out.reshape(B, C, H, W)
